# revision 1
# baseline (speedup 1.0000x reference)
"""Multi-head attention (B=2, N=2048, C=1024, H=16, D=64) on 8 TRN2 NeuronCores.

Sharding: 2 heads per core (tensor parallel over num_heads), both batch
elements processed on every core.  Each core computes q/k/v projections for
its 2 heads, full attention for those heads, and a partial output projection
(row-parallel over w_proj); the host sums the 8 partial outputs and adds the
bias.

Device-side dataflow per core:
  qkv:   qT/kT/vT [dpair=128, N] from xT tiles (c on partitions, f32r
         matmuls at full PE rate), accumulating over 8 c-tiles of 128.
         q/k are evacuated to bf16 with each head's 64 d-rows duplicated
         onto both partition halves, so score matmuls for two m-tiles can
         row-pack the PE array (rows 0:64 and 64:128 run concurrently).
  v:     vT -> bf16 -> PE transpose (128x128 tiles) -> vo tiles [m, d].
  attn:  per head, per m-tile pair: scores^T = kT_tile.T @ qT (K=64),
         exp via ACT (scale=1/8 folded in; no max-subtraction needed:
         logits are O(3) so fp32 exp is exact), writing bf16 E^T tiles;
         AV accumulation over m into PSUM, with a col-tiled ones matmul
         (cols 64:128 of the array) producing softmax denominators
         concurrently with the V matmul.
  norm:  reciprocal + cross-partition multiply into ocatT (f32r).
  proj:  y_partial[n, :] = ocatT.T @ w_projT, K=128 one-shot f32r matmuls.
"""

import sys

sys.path.insert(0, "/opt/trn_rl_repo")

import numpy as np

import concourse.bass as bass
import concourse.mybir as mybir
import concourse.tile as tile
from concourse import bacc
from concourse.bass_utils import run_bass_kernel_spmd
from concourse.masks import make_identity

F32 = mybir.dt.float32
F32R = mybir.dt.float32r
BF16 = mybir.dt.bfloat16
AF = mybir.ActivationFunctionType

B = 2
N = 2048
C = 1024
H = 16
D = 64
NCORES = 8
HPC = H // NCORES          # heads per core = 2
CT = C // 128              # c tiles = 8
NT = N // 128              # n/m tiles = 16
NCH = N // 512             # 512-wide n chunks = 4
SCALE = float(D) ** -0.5


def _build():
    nc = bacc.Bacc("TRN2")
    xT = nc.dram_tensor("xT", [B, C, N], F32R, kind="ExternalInput")
    wqkT = nc.dram_tensor("wqkT", [CT, 128, 256], F32R, kind="ExternalInput")
    wvT = nc.dram_tensor("wvT", [CT, 128, 128], F32R, kind="ExternalInput")
    wpT = nc.dram_tensor("wpT", [128, C], F32R, kind="ExternalInput")
    y = nc.dram_tensor("y", [B, N, C], F32, kind="ExternalOutput")

    with tile.TileContext(nc) as tc:
        with tc.tile_pool(name="consts", bufs=1) as consts, \
             tc.tile_pool(name="xt", bufs=8) as xt_pool, \
             tc.tile_pool(name="qk", bufs=8) as qk_pool, \
             tc.tile_pool(name="vt", bufs=2) as vt_pool, \
             tc.tile_pool(name="vo", bufs=2) as vo_pool, \
             tc.tile_pool(name="et", bufs=4) as et_pool, \
             tc.tile_pool(name="oc", bufs=2) as oc_pool, \
             tc.tile_pool(name="rec", bufs=2) as rec_pool, \
             tc.tile_pool(name="yo", bufs=4) as yo_pool, \
             tc.tile_pool(name="pbig", bufs=2, space="PSUM") as pbig, \
             tc.tile_pool(name="pav", bufs=4, space="PSUM") as pav:

            wqk_sb = consts.tile([128, CT, 256], F32R)
            wv_sb = consts.tile([128, CT, 128], F32R)
            wp_sb = consts.tile([128, C], F32R)
            ones_bf = consts.tile([128, 64], BF16)
            ident_bf = consts.tile([128, 128], BF16)
            nc.sync.dma_start(out=wqk_sb, in_=wqkT[:, :, :].rearrange("t p o -> p t o"))
            nc.sync.dma_start(out=wv_sb, in_=wvT[:, :, :].rearrange("t p o -> p t o"))
            nc.sync.dma_start(out=wp_sb, in_=wpT[:, :])
            nc.vector.memset(ones_bf, 1.0)
            make_identity(nc, ident_bf[:, :])

            for b in range(B):
                # ---- load xT tiles (c on partitions) ----
                xt = []
                for ct in range(CT):
                    t = xt_pool.tile([128, N], F32R, tag="xt", name=f"xt_{b}_{ct}")
                    nc.sync.dma_start(out=t, in_=xT[b, ct * 128:(ct + 1) * 128, :])
                    xt.append(t)

                # ---- q/k projections into duplicated-partition bf16 layout ----
                qd = [qk_pool.tile([128, N], BF16, tag="qk", name=f"qd_{b}_{h}")
                      for h in range(HPC)]
                kd = [qk_pool.tile([128, N], BF16, tag="qk", name=f"kd_{b}_{h}")
                      for h in range(HPC)]
                for ot, dsts in ((0, qd), (1, kd)):
                    for nch in range(NCH):
                        ps = pbig.tile([128, 512], F32, tag="pb",
                                       name=f"ps_{b}_{ot}_{nch}")
                        for ct in range(CT):
                            nc.tensor.matmul(
                                ps[:, :],
                                wqk_sb[:, ct, ot * 128:(ot + 1) * 128],
                                xt[ct][:, nch * 512:(nch + 1) * 512],
                                start=(ct == 0), stop=(ct == CT - 1),
                            )
                        sl = slice(nch * 512, (nch + 1) * 512)
                        for h in range(HPC):
                            src = ps[h * 64:(h + 1) * 64, :]
                            cp0 = nc.scalar.copy if b == 0 else nc.vector.tensor_copy
                            cp0(dsts[h][0:64, sl], src)
                            nc.vector.tensor_copy(dsts[h][64:128, sl], src)

                # ---- v projection (vT) + PE transpose to vo [m, d] ----
                vt_bf = vt_pool.tile([128, N], BF16, tag="vt", name=f"vt_{b}")
                for nch in range(NCH):
                    ps = pbig.tile([128, 512], F32, tag="pb", name=f"psv_{b}_{nch}")
                    for ct in range(CT):
                        nc.tensor.matmul(
                            ps[:, :],
                            wv_sb[:, ct, :],
                            xt[ct][:, nch * 512:(nch + 1) * 512],
                            start=(ct == 0), stop=(ct == CT - 1),
                        )
                    (nc.scalar.copy if b == 0 else nc.vector.tensor_copy)(
                        vt_bf[:, nch * 512:(nch + 1) * 512], ps[:, :])
                # vo layout per m-tile: [V_h0 (64) | ones (64) | V_h1 (64)] so each
                # head's AV stationary operand is a contiguous [V|ones] 128-col slab
                # (h0: cols 0:128 -> out = [O'; denom], h1: cols 64:192 -> [denom; O'])
                vo = vo_pool.tile([128, NT, 192], BF16, tag="vo", name=f"vo_{b}")
                nc.vector.memset(vo[:, :, 64:128], 1.0)
                for mt in range(NT):
                    tp = pbig.tile([128, 128], BF16, tag="pb", name=f"tp_{b}_{mt}")
                    nc.tensor.transpose(
                        tp[:, :], vt_bf[:, mt * 128:(mt + 1) * 128], ident_bf[:, :])
                    nc.vector.tensor_copy(vo[:, mt, 0:64], tp[:, 0:64])
                    nc.vector.tensor_copy(vo[:, mt, 128:192], tp[:, 64:128])

                oc_sb = oc_pool.tile([128, N], F32R, tag="oc", name=f"oc_{b}")

                # ---- attention per head (m-tiles processed in row-packed pairs) ----
                for hl in range(HPC):
                    hs = hl * 64
                    qdh, kdh = qd[hl], kd[hl]
                    avs = [pav.tile([128, 512], F32, tag="av",
                                    name=f"av_{b}_{hl}_{i}") for i in range(NCH)]
                    for j in range(NT // 2):
                        mA, mB = 2 * j, 2 * j + 1
                        # E^T for the pair, packed per n-quarter:
                        # et[:, q, 0:512] = E(mA, q-chunk), et[:, q, 512:1024] = E(mB, q-chunk)
                        et = et_pool.tile([128, NCH, 1024], BF16, tag="et",
                                          name=f"et_{b}_{hl}_{j}")
                        for q in range(NCH):
                            qof = q * 512
                            s = pbig.tile([128, 1024], F32, tag="pb",
                                          name=f"s_{b}_{hl}_{j}_{q}")
                            nc.tensor.matmul(
                                s[:, 0:512],
                                kdh[0:64, mA * 128:(mA + 1) * 128],
                                qdh[0:64, qof:qof + 512],
                                start=True, stop=True,
                            )
                            nc.tensor.matmul(
                                s[:, 512:1024],
                                kdh[64:128, mB * 128:(mB + 1) * 128],
                                qdh[64:128, qof:qof + 512],
                                start=True, stop=True,
                            )
                            nc.scalar.activation(out=et[:, q, :], in_=s[:, :],
                                                 func=AF.Exp, scale=SCALE)
                        for q in range(NCH):
                            for m_, eof in ((mA, 0), (mB, 512)):
                                nc.tensor.matmul(
                                    avs[q][:, :],
                                    vo[:, m_, hs:hs + 128],
                                    et[:, q, eof:eof + 512],
                                    start=(m_ == 0), stop=(m_ == NT - 1),
                                )
                    # h0: out partitions 0:64 = O', 64:128 = denom; h1 swapped
                    osl = slice(0, 64) if hl == 0 else slice(64, 128)
                    dsl = slice(64, 128) if hl == 0 else slice(0, 64)
                    for qq in range(NCH):
                        rec = rec_pool.tile([128, 512], F32, tag="rec",
                                            name=f"rec_{b}_{hl}_{qq}")
                        nc.vector.reciprocal(rec[dsl, :], avs[qq][dsl, :])
                        nc.vector.tensor_mul(
                            oc_sb[hs:hs + 64, qq * 512:(qq + 1) * 512],
                            avs[qq][osl, :],
                            rec[dsl, :],
                        )

                # ---- output projection (partial over this core's c-block) ----
                for nt in range(NT):
                    for och in range(2):
                        pp = pav.tile([128, 512], F32, tag="av",
                                      name=f"pp_{b}_{nt}_{och}")
                        nc.tensor.matmul(
                            pp[:, :],
                            oc_sb[:, nt * 128:(nt + 1) * 128],
                            wp_sb[:, och * 512:(och + 1) * 512],
                            start=True, stop=True,
                        )
                        ysb = yo_pool.tile([128, 512], F32, tag="yo",
                                           name=f"ysb_{b}_{nt}_{och}")
                        (nc.scalar.copy if (b == 1 and (nt + och) % 2 == 0)
                         else nc.vector.tensor_copy)(ysb[:, :], pp[:, :])
                        nc.sync.dma_start(
                            out=y[b, nt * 128:(nt + 1) * 128,
                                  och * 512:(och + 1) * 512],
                            in_=ysb[:, :],
                        )
    nc.finalize()
    return nc


_NC = None


def _get_nc():
    global _NC
    if _NC is None:
        _NC = _build()
    return _NC


def _make_in_maps(x, w_qkv):
    xT = np.ascontiguousarray(x.transpose(0, 2, 1)).astype(np.float32)
    in_maps = []
    for core in range(NCORES):
        h0 = core * HPC
        rows = np.concatenate(
            [np.arange(h * D, (h + 1) * D) for h in range(h0, h0 + HPC)]
        )
        wqk = np.concatenate([w_qkv[rows, :], w_qkv[C + rows, :]], axis=0)  # [256, 1024]
        wqkT = np.ascontiguousarray(wqk.T).reshape(CT, 128, 256)
        wvT = np.ascontiguousarray(w_qkv[2 * C + rows, :].T).reshape(CT, 128, 128)
        in_maps.append({"xT": xT, "wqkT": wqkT, "wvT": wvT})
    return in_maps


def kernel(x, w_qkv, w_proj, b_proj):
    x = np.asarray(x, dtype=np.float32)
    w_qkv = np.asarray(w_qkv, dtype=np.float32)
    w_proj = np.asarray(w_proj, dtype=np.float32)
    b_proj = np.asarray(b_proj, dtype=np.float32)

    in_maps = _make_in_maps(x, w_qkv)
    for core in range(NCORES):
        h0 = core * HPC
        cols = np.arange(h0 * D, (h0 + HPC) * D)
        in_maps[core]["wpT"] = np.ascontiguousarray(w_proj[:, cols].T)  # [128, 1024]

    nc = _get_nc()
    res = run_bass_kernel_spmd(nc, in_maps, core_ids=list(range(NCORES)))
    out = np.zeros((B, N, C), dtype=np.float32)
    for core in range(NCORES):
        out += res.results[core]["y"]
    out += b_proj
    return out



# revision 2
# speedup vs baseline: 1.2883x; 1.2883x over previous
"""Multi-head attention (B=2, N=2048, C=1024, H=16, D=64) on 8 TRN2 NeuronCores.

Sharding: tensor-parallel over heads (2 heads/core), both batches on every
core; output projection row-parallel over the core's 128 attention-output
channels; host sums the 8 partial y tensors and adds the bias.

Per-core schedule (static, engine-queue aware):
  - Act engine runs ONLY the 128 softmax-exp activations ([128,1024] each) —
    it is the binding stream inside attention.
  - PE: bf16 qkv projection, scores (fp8 DoubleRow with dithered dual
    quantization: both DR slots carry differently-scaled fp8 copies of q/k so
    their products average, halving quantization noise variance at zero PE
    cost), bf16 AV with the packed [V|ones] stationary trick producing softmax
    denominators for free, bf16 output projection.
  - DVE: all PSUM evacuations (q/k -> fp8 with dither scales, v -> bf16),
    V-transpose PSUM evac into vo slabs, reciprocal+multiply normalization,
    proj PSUM evac.
  - Engine queues execute in order, so qkv/transpose work of batch b+1 and the
    projection of batch b are emitted as fine-grained filler quanta INSIDE the
    attention j-loops to keep the PE busy while Act chews through exps.
"""

import sys

sys.path.insert(0, "/opt/trn_rl_repo")

import numpy as np
import ml_dtypes

import concourse.mybir as mybir
import concourse.tile as tile
from concourse import bacc
from concourse.bass_utils import run_bass_kernel_spmd
from concourse.masks import make_identity

F32 = mybir.dt.float32
BF16 = mybir.dt.bfloat16
FP8 = mybir.dt.float8e4
AF = mybir.ActivationFunctionType
DR = mybir.MatmulPerfMode.DoubleRow

BF = ml_dtypes.bfloat16

B = 2
N = 2048
C = 1024
H = 16
D = 64
NCORES = 8
HPC = H // NCORES          # heads per core = 2
CT = C // 128              # contraction tiles = 8
NT = N // 128              # m tiles = 16
NCH = N // 512             # 512-wide n chunks = 4
SCALE = float(D) ** -0.5

SCORES_FP8 = True          # dithered fp8 DoubleRow scores vs plain bf16
DITHER_C = 1.3


def _build():
    nc = bacc.Bacc("TRN2")
    xTr = nc.dram_tensor("xTr", [B, 128, CT, N], BF16, kind="ExternalInput")
    wT = nc.dram_tensor("wT", [128, CT, 3, 128], BF16, kind="ExternalInput")
    wpT = nc.dram_tensor("wpT", [128, C], BF16, kind="ExternalInput")
    y = nc.dram_tensor("y", [B, N, C], F32, kind="ExternalOutput")

    with tile.TileContext(nc) as tc:
        with tc.tile_pool(name="consts", bufs=1) as consts, \
             tc.tile_pool(name="xt", bufs=2) as xt_pool, \
             tc.tile_pool(name="qk", bufs=2) as qk_pool, \
             tc.tile_pool(name="vt", bufs=2) as vt_pool, \
             tc.tile_pool(name="vo", bufs=2) as vo_pool, \
             tc.tile_pool(name="et", bufs=4) as et_pool, \
             tc.tile_pool(name="oc", bufs=2) as oc_pool, \
             tc.tile_pool(name="rec", bufs=2) as rec_pool, \
             tc.tile_pool(name="yo", bufs=4) as yo_pool, \
             tc.tile_pool(name="ps_s", bufs=2, space="PSUM") as ps_s, \
             tc.tile_pool(name="ps_av", bufs=2, space="PSUM") as ps_av, \
             tc.tile_pool(name="ps_mm", bufs=2, space="PSUM") as ps_mm:

            wt_sb = consts.tile([128, CT, 3, 128], BF16)
            wp_sb = consts.tile([128, C], BF16)
            ident_bf = consts.tile([128, 128], BF16)
            nc.sync.dma_start(out=wt_sb, in_=wT[:, :, :, :])
            nc.sync.dma_start(out=wp_sb, in_=wpT[:, :])
            for b in range(B):
                for cc in range(4):
                    pass
            make_identity(nc, ident_bf[:, :])

            # per-batch state tiles
            st = {}
            for b in range(B):
                st[b] = {}

            def load_x(b):
                t = xt_pool.tile([128, CT, N], BF16, tag="xt", name=f"xt{b}")
                for cc in range(4):
                    nc.sync.dma_start(
                        out=t[:, 2 * cc:2 * cc + 2, :],
                        in_=xTr[b, :, 2 * cc:2 * cc + 2, :])
                st[b]["xt"] = t

            def alloc_batch(b):
                if SCORES_FP8:
                    st[b]["qf"] = qk_pool.tile([128, 2, N], FP8, tag="qf",
                                               name=f"qf{b}")
                    st[b]["kf"] = qk_pool.tile([128, 2, N], FP8, tag="kf",
                                               name=f"kf{b}")
                else:
                    st[b]["qf"] = qk_pool.tile([128, N], BF16, tag="qf",
                                               name=f"qf{b}")
                    st[b]["kf"] = qk_pool.tile([128, N], BF16, tag="kf",
                                               name=f"kf{b}")
                st[b]["vt"] = vt_pool.tile([128, N], BF16, tag="vt", name=f"vt{b}")
                st[b]["vo"] = vo_pool.tile([128, NT, 192], BF16, tag="vo",
                                           name=f"vo{b}")
                st[b]["oc"] = oc_pool.tile([128, N], BF16, tag="oc", name=f"oc{b}")
                nc.vector.memset(st[b]["vo"][:, :, 64:128], 1.0)

            def qkv_gen(b):
                """bf16 qkv projection + fp8/bf16 evacuation; yields ~2-mm quanta."""
                xt = st[b]["xt"]
                qf, kf, vt = st[b]["qf"], st[b]["kf"], st[b]["vt"]
                for nch in range(NCH):
                    csl = slice(nch * 512, (nch + 1) * 512)
                    for si in range(3):
                        ps = ps_mm.tile([128, 512], F32, tag="mm",
                                        name=f"qkv{b}_{nch}_{si}")
                        for ct in range(CT):
                            nc.tensor.matmul(
                                ps[:, :],
                                wt_sb[:, ct, si, :],
                                xt[:, ct, csl],
                                start=(ct == 0), stop=(ct == CT - 1),
                            )
                            if ct % 4 == 3:
                                yield
                        if si == 2:
                            nc.vector.tensor_copy(vt[:, csl], ps[:, :])
                        elif SCORES_FP8:
                            dst = qf if si == 0 else kf
                            ca = DITHER_C if si == 0 else 1.0 / DITHER_C
                            nc.vector.tensor_scalar_mul(dst[:, 0, csl], ps[:, :], ca)
                            nc.vector.tensor_scalar_mul(dst[:, 1, csl], ps[:, :],
                                                        1.0 / ca)
                        else:
                            dst = qf if si == 0 else kf
                            nc.vector.tensor_copy(dst[:, csl], ps[:, :])
                        yield

            def tpvo_gen(b):
                """PE-transpose V and build [V_h0 | ones | V_h1] slabs."""
                vt, vo = st[b]["vt"], st[b]["vo"]
                for mt in range(NT):
                    tp = ps_mm.tile([128, 128], BF16, tag="mm", name=f"tp{b}_{mt}")
                    nc.tensor.transpose(
                        tp[:, :], vt[:, mt * 128:(mt + 1) * 128], ident_bf[:, :])
                    nc.vector.tensor_copy(vo[:, mt, 0:64], tp[:, 0:64])
                    nc.vector.tensor_copy(vo[:, mt, 128:192], tp[:, 64:128])
                    yield

            def proj_gen(b):
                """Output projection + evac + store."""
                oc = st[b]["oc"]
                for nt in range(NT):
                    for och in range(2):
                        pp = ps_av.tile([128, 512], F32, tag="av",
                                        name=f"pp{b}_{nt}_{och}")
                        nc.tensor.matmul(
                            pp[:, :],
                            oc[:, nt * 128:(nt + 1) * 128],
                            wp_sb[:, och * 512:(och + 1) * 512],
                            start=True, stop=True,
                        )
                        ysb = yo_pool.tile([128, 512], F32, tag="yo",
                                           name=f"ysb{b}_{nt}_{och}")
                        nc.vector.tensor_copy(ysb[:, :], pp[:, :])
                        nc.sync.dma_start(
                            out=y[b, nt * 128:(nt + 1) * 128,
                                  och * 512:(och + 1) * 512],
                            in_=ysb[:, :],
                        )
                        yield

            def take(fillers, n):
                """Emit up to n quanta from the filler generator chain."""
                done = 0
                while done < n and fillers:
                    try:
                        next(fillers[0])
                        done += 1
                    except StopIteration:
                        fillers.pop(0)
                return done

            def attn_block(b, hl, qp, fillers, quota):
                hs = hl * 64
                qf, kf, vo, oc = (st[b][k] for k in ("qf", "kf", "vo", "oc"))
                chunks = (2 * qp, 2 * qp + 1)
                avs = [ps_av.tile([128, 512], F32, tag="av",
                                  name=f"av{b}_{hl}_{qp}_{ci}")
                       for ci in range(2)]
                prev = None
                for j in range(NT // 2):
                    cur = []
                    for ci, ch in enumerate(chunks):
                        s = ps_s.tile([128, 1024], F32, tag="s",
                                      name=f"s{b}_{hl}_{qp}_{j}_{ci}")
                        for half, mt in ((0, 2 * j), (1, 2 * j + 1)):
                            osl = slice(half * 512, (half + 1) * 512)
                            if SCORES_FP8:
                                nc.tensor.matmul(
                                    s[:, osl],
                                    kf[hs:hs + 64, :, mt * 128:(mt + 1) * 128],
                                    qf[hs:hs + 64, :, ch * 512:(ch + 1) * 512],
                                    start=True, stop=True, perf_mode=DR,
                                )
                            else:
                                nc.tensor.matmul(
                                    s[:, osl],
                                    kf[hs:hs + 64, mt * 128:(mt + 1) * 128],
                                    qf[hs:hs + 64, ch * 512:(ch + 1) * 512],
                                    start=True, stop=True,
                                )
                        et_t = et_pool.tile([128, 1024], BF16, tag="et",
                                            name=f"et{b}_{hl}_{qp}_{j}_{ci}")
                        nc.scalar.activation(
                            out=et_t[:, :], in_=s[:, :], func=AF.Exp,
                            scale=SCALE / 2.0 if SCORES_FP8 else SCALE)
                        cur.append((ci, et_t))
                    take(fillers, quota)
                    if prev is not None:
                        pj, pcur = prev
                        for ci, et_t in pcur:
                            for half, mt in ((0, 2 * pj), (1, 2 * pj + 1)):
                                nc.tensor.matmul(
                                    avs[ci][:, :],
                                    vo[:, mt, hs:hs + 128],
                                    et_t[:, half * 512:(half + 1) * 512],
                                    start=(mt == 0), stop=(mt == NT - 1),
                                )
                    prev = (j, cur)
                pj, pcur = prev
                for ci, et_t in pcur:
                    for half, mt in ((0, 2 * pj), (1, 2 * pj + 1)):
                        nc.tensor.matmul(
                            avs[ci][:, :],
                            vo[:, mt, hs:hs + 128],
                            et_t[:, half * 512:(half + 1) * 512],
                            start=(mt == 0), stop=(mt == NT - 1),
                        )
                # normalization: oc[head rows, chunk] = O' / denom
                osl = slice(0, 64) if hl == 0 else slice(64, 128)
                dsl = slice(64, 128) if hl == 0 else slice(0, 64)
                for ci, ch in enumerate(chunks):
                    rec = rec_pool.tile([128, 512], F32, tag="rec",
                                        name=f"rec{b}_{hl}_{qp}_{ci}")
                    nc.vector.reciprocal(rec[dsl, :], avs[ci][dsl, :])
                    nc.vector.tensor_mul(
                        oc[hs:hs + 64, ch * 512:(ch + 1) * 512],
                        avs[ci][osl, :],
                        rec[dsl, :],
                    )

            # ---------------- static schedule ----------------
            load_x(0)
            load_x(1)
            alloc_batch(0)

            # prologue: batch-0 qkv + transpose fully emitted
            f0 = [qkv_gen(0), tpvo_gen(0)]
            while take(f0, 64):
                pass

            alloc_batch(1)
            fill1 = [qkv_gen(1), tpvo_gen(1)]
            for hl in range(HPC):
                for qp in range(2):
                    attn_block(0, hl, qp, fill1, quota=2)
            while take(fill1, 64):
                pass

            fill2 = [proj_gen(0)]
            for hl in range(HPC):
                for qp in range(2):
                    attn_block(1, hl, qp, fill2, quota=1)
            while take(fill2, 64):
                pass
            for _ in proj_gen(1):
                pass
    nc.finalize()
    return nc


_NC = None


def _get_nc():
    global _NC
    if _NC is None:
        _NC = _build()
    return _NC


def _prep_shared(x):
    # x [B, N, C] -> xTr [B, 128, CT, N] bf16 (c = ct*128 + p)
    xT = x.transpose(0, 2, 1).reshape(B, CT, 128, N).transpose(0, 2, 1, 3)
    return np.ascontiguousarray(xT).astype(BF)


def kernel(x, w_qkv, w_proj, b_proj):
    x = np.asarray(x, dtype=np.float32)
    w_qkv = np.asarray(w_qkv, dtype=np.float32)
    w_proj = np.asarray(w_proj, dtype=np.float32)
    b_proj = np.asarray(b_proj, dtype=np.float32)

    xTr = _prep_shared(x)
    in_maps = []
    for core in range(NCORES):
        h0 = core * HPC
        rows = np.concatenate(
            [np.arange(h * D, (h + 1) * D) for h in range(h0, h0 + HPC)])
        wsel = np.concatenate(
            [w_qkv[rows, :], w_qkv[C + rows, :], w_qkv[2 * C + rows, :]], axis=0)
        # [384, C] -> [C, 384] -> [CT, 128, 3, 128] -> [128, CT, 3, 128]
        wT = wsel.T.reshape(CT, 128, 3, 128).transpose(1, 0, 2, 3)
        wT = np.ascontiguousarray(wT).astype(BF)
        cols = np.arange(h0 * D, (h0 + HPC) * D)
        wpT = np.ascontiguousarray(w_proj[:, cols].T).astype(BF)
        in_maps.append({"xTr": xTr, "wT": wT, "wpT": wpT})

    nc = _get_nc()
    res = run_bass_kernel_spmd(nc, in_maps, core_ids=list(range(NCORES)))
    out = np.zeros((B, N, C), dtype=np.float32)
    for core in range(NCORES):
        out += res.results[core]["y"]
    out += b_proj
    return out


# revision 6
# speedup vs baseline: 1.5628x; 1.2131x over previous
"""Multi-head attention (B=2, N=2048, C=1024, H=16, D=64) on 8 TRN2 NeuronCores.

Sharding: tensor-parallel over heads (2 heads/core), both batches on every
core; output projection row-parallel over the core's 128 attention-output
channels; host sums the 8 partial y tensors and adds the bias.

Per-core schedule (static, engine-queue aware):
  - Act engine runs ONLY the 128 softmax-exp activations ([128,1024] each).
  - PE: bf16 qkv projection, scores as fp8 DoubleRow with dithered dual
    quantization (both DR slots carry differently-scaled fp8 copies of q/k;
    their products average, halving quantization noise at zero PE cost),
    bf16 AV with the packed [V|ones] stationary trick producing softmax
    denominators for free, bf16 output projection.
  - DVE: all PSUM evacuations, reciprocal+multiply normalization (deferred
    into the next attention block to avoid head-of-line blocking).
  - Engine queues execute in order, so qkv/transpose/proj work is emitted as
    fine-grained filler quanta INSIDE the attention j-loops.
"""

import sys

sys.path.insert(0, "/opt/trn_rl_repo")

import numpy as np
import ml_dtypes

import concourse.mybir as mybir
import concourse.tile as tile
from concourse import bacc
from concourse.bass_utils import run_bass_kernel_spmd
from concourse.masks import make_identity

F32 = mybir.dt.float32
BF16 = mybir.dt.bfloat16
FP8 = mybir.dt.float8e4
AF = mybir.ActivationFunctionType
DR = mybir.MatmulPerfMode.DoubleRow

BF = ml_dtypes.bfloat16

B = 2
N = 2048
C = 1024
H = 16
D = 64
NCORES = 8
HPC = H // NCORES          # heads per core = 2
CT = C // 128              # contraction tiles = 8
NT = N // 128              # m tiles = 16
NCH = N // 512             # 512-wide n chunks = 4
SCALE = float(D) ** -0.5

SCORES_FP8 = True          # dithered fp8 DoubleRow scores vs plain bf16
DITHER_C = 1.3


def _build():
    nc = bacc.Bacc("TRN2")
    xTr = nc.dram_tensor("xTr", [B, 128, CT, N], BF16, kind="ExternalInput")
    wT = nc.dram_tensor("wT", [128, CT, 3, 128], BF16, kind="ExternalInput")
    wpT = nc.dram_tensor("wpT", [128, C], BF16, kind="ExternalInput")
    y = nc.dram_tensor("y", [B, N, C], F32, kind="ExternalOutput")

    with tile.TileContext(nc) as tc:
        with tc.tile_pool(name="consts", bufs=1) as consts, \
             tc.tile_pool(name="xt", bufs=2) as xt_pool, \
             tc.tile_pool(name="qk", bufs=2) as qk_pool, \
             tc.tile_pool(name="vt", bufs=2) as vt_pool, \
             tc.tile_pool(name="vo", bufs=2) as vo_pool, \
             tc.tile_pool(name="et", bufs=4) as et_pool, \
             tc.tile_pool(name="oc", bufs=2) as oc_pool, \
             tc.tile_pool(name="rec", bufs=2) as rec_pool, \
             tc.tile_pool(name="yo", bufs=4) as yo_pool, \
             tc.tile_pool(name="ps_s", bufs=2, space="PSUM") as ps_s, \
             tc.tile_pool(name="ps_av", bufs=2, space="PSUM") as ps_av, \
             tc.tile_pool(name="ps_mm", bufs=2, space="PSUM") as ps_mm:

            wt_sb = consts.tile([128, CT, 3, 128], BF16)
            wp_sb = consts.tile([128, C], BF16)
            ident_bf = consts.tile([128, 128], BF16)
            # split weight load so the first qkv matmul unblocks early
            for cc in range(4):
                nc.sync.dma_start(out=wt_sb[:, 2 * cc:2 * cc + 2, :, :],
                                  in_=wT[:, 2 * cc:2 * cc + 2, :, :])
            nc.sync.dma_start(out=wp_sb, in_=wpT[:, :])
            make_identity(nc, ident_bf[:, :])

            st = {b: {} for b in range(B)}

            def load_x(b):
                t = xt_pool.tile([128, CT, N], BF16, tag="xt", name=f"xt{b}")
                for cc in range(CT):
                    nc.sync.dma_start(out=t[:, cc:cc + 1, :],
                                      in_=xTr[b, :, cc:cc + 1, :])
                st[b]["xt"] = t

            def alloc_batch(b):
                if SCORES_FP8:
                    st[b]["qf"] = qk_pool.tile([128, 2, N], FP8, tag="qf",
                                               name=f"qf{b}")
                    st[b]["kf"] = qk_pool.tile([128, 2, N], FP8, tag="kf",
                                               name=f"kf{b}")
                else:
                    st[b]["qf"] = qk_pool.tile([128, N], BF16, tag="qf",
                                               name=f"qf{b}")
                    st[b]["kf"] = qk_pool.tile([128, N], BF16, tag="kf",
                                               name=f"kf{b}")
                st[b]["vt"] = vt_pool.tile([128, N], BF16, tag="vt", name=f"vt{b}")
                st[b]["vo"] = vo_pool.tile([128, NT, 192], BF16, tag="vo",
                                           name=f"vo{b}")
                st[b]["oc"] = oc_pool.tile([128, N], BF16, tag="oc", name=f"oc{b}")
                nc.vector.memset(st[b]["vo"][:, :, 64:128], 1.0)

            prog = set()

            def qkv_part(b, parts):
                """Emit qkv projection for (si, nch) pairs; yield every 2 mms."""
                xt = st[b]["xt"]
                qf, kf, vt = st[b]["qf"], st[b]["kf"], st[b]["vt"]
                for si, nch in parts:
                    csl = slice(nch * 512, (nch + 1) * 512)
                    ps = ps_mm.tile([128, 512], F32, tag="mm",
                                    name=f"qkv{b}_{nch}_{si}")
                    for ct in range(CT):
                        nc.tensor.matmul(
                            ps[:, :],
                            wt_sb[:, ct, si, :],
                            xt[:, ct, csl],
                            start=(ct == 0), stop=(ct == CT - 1),
                        )
                        if ct % 2 == 1:
                            yield
                    if si == 2:
                        nc.vector.tensor_copy(vt[:, csl], ps[:, :])
                    elif SCORES_FP8:
                        dst = qf if si == 0 else kf
                        ca = DITHER_C if si == 0 else 1.0 / DITHER_C
                        nc.vector.tensor_scalar_mul(dst[:, 0, csl], ps[:, :], ca)
                        nc.vector.tensor_scalar_mul(dst[:, 1, csl], ps[:, :],
                                                    1.0 / ca)
                    else:
                        dst = qf if si == 0 else kf
                        nc.vector.tensor_copy(dst[:, csl], ps[:, :])
                    prog.add(("qkv"[si], b, nch))
                    yield

            def tpvo_part(b, mts):
                """PE-transpose V m-tiles into [V_h0 | ones | V_h1] slabs."""
                vt, vo = st[b]["vt"], st[b]["vo"]
                for mt in mts:
                    tp = ps_mm.tile([128, 128], BF16, tag="mm", name=f"tp{b}_{mt}")
                    nc.tensor.transpose(
                        tp[:, :], vt[:, mt * 128:(mt + 1) * 128], ident_bf[:, :])
                    nc.vector.tensor_copy(vo[:, mt, 0:64], tp[:, 0:64])
                    nc.vector.tensor_copy(vo[:, mt, 128:192], tp[:, 64:128])
                    prog.add(("vo", b, mt))
                    yield

            def proj_part(b, nts):
                """Output projection + evac + store for the given n-tiles."""
                oc = st[b]["oc"]
                for nt in nts:
                    for och in range(2):
                        pp = ps_mm.tile([128, 512], F32, tag="mm",
                                        name=f"pp{b}_{nt}_{och}")
                        nc.tensor.matmul(
                            pp[:, :],
                            oc[:, nt * 128:(nt + 1) * 128],
                            wp_sb[:, och * 512:(och + 1) * 512],
                            start=True, stop=True,
                        )
                        ysb = yo_pool.tile([128, 512], F32, tag="yo",
                                           name=f"ysb{b}_{nt}_{och}")
                        nc.vector.tensor_copy(ysb[:, :], pp[:, :])
                        nc.sync.dma_start(
                            out=y[b, nt * 128:(nt + 1) * 128,
                                  och * 512:(och + 1) * 512],
                            in_=ysb[:, :],
                        )
                        yield

            def take(fillers, n):
                done = 0
                while done < n and fillers:
                    try:
                        next(fillers[0])
                        done += 1
                    except StopIteration:
                        fillers.pop(0)
                return done

            def need(fillers, key):
                while key not in prog:
                    if not take(fillers, 1):
                        raise RuntimeError(f"unreachable dep {key}")

            def attn_block(b, hl, qp, fillers, quota, post):
                """One (batch, head, chunk-pair) attention block.

                `post` (the previous block's deferred normalizations) is
                emitted at slot j==0 so the DVE never parks on an unfinished
                AV accumulation.  Data dependencies on filler-produced tiles
                are enforced by need() pulls (emission order == dataflow).
                """
                hs = hl * 64
                qf, kf, vo, oc = (st[b][k] for k in ("qf", "kf", "vo", "oc"))
                chunks = (2 * qp, 2 * qp + 1)
                need(fillers, ("q", b, 2 * qp))
                need(fillers, ("q", b, 2 * qp + 1))
                avs = [ps_av.tile([128, 512], F32, tag="av",
                                  name=f"av{b}_{hl}_{qp}_{ci}")
                       for ci in range(2)]
                prev = None

                def emit_av(j, ets):
                    for ci, et_t in ets:
                        for half, mt in ((0, 2 * j), (1, 2 * j + 1)):
                            nc.tensor.matmul(
                                avs[ci][:, :],
                                vo[:, mt, hs:hs + 128],
                                et_t[:, half * 512:(half + 1) * 512],
                                start=(mt == 0), stop=(mt == NT - 1),
                            )

                for j in range(NT // 2):
                    need(fillers, ("k", b, (2 * j + 1) // 4))
                    cur = []
                    for ci, ch in enumerate(chunks):
                        s = ps_s.tile([128, 1024], F32, tag="s",
                                      name=f"s{b}_{hl}_{qp}_{j}_{ci}")
                        for half, mt in ((0, 2 * j), (1, 2 * j + 1)):
                            osl = slice(half * 512, (half + 1) * 512)
                            if SCORES_FP8:
                                nc.tensor.matmul(
                                    s[:, osl],
                                    kf[hs:hs + 64, :, mt * 128:(mt + 1) * 128],
                                    qf[hs:hs + 64, :, ch * 512:(ch + 1) * 512],
                                    start=True, stop=True, perf_mode=DR,
                                )
                            else:
                                nc.tensor.matmul(
                                    s[:, osl],
                                    kf[hs:hs + 64, mt * 128:(mt + 1) * 128],
                                    qf[hs:hs + 64, ch * 512:(ch + 1) * 512],
                                    start=True, stop=True,
                                )
                        et_t = et_pool.tile([128, 1024], BF16, tag="et",
                                            name=f"et{b}_{hl}_{qp}_{j}_{ci}")
                        nc.scalar.activation(
                            out=et_t[:, :], in_=s[:, :], func=AF.Exp,
                            scale=SCALE / 2.0 if SCORES_FP8 else SCALE)
                        cur.append((ci, et_t))
                    if j == 0:
                        for fn in post:
                            fn()
                        post = []
                    take(fillers, quota)
                    if prev is not None:
                        need(fillers, ("vo", b, 2 * prev[0] + 1))
                        emit_av(*prev)
                    prev = (j, cur)
                need(fillers, ("vo", b, 2 * prev[0] + 1))
                emit_av(*prev)

                osl = slice(0, 64) if hl == 0 else slice(64, 128)
                dsl = slice(64, 128) if hl == 0 else slice(0, 64)

                def norm(ci, ch):
                    def fn():
                        rec = rec_pool.tile([128, 512], F32, tag="rec",
                                            name=f"rec{b}_{hl}_{qp}_{ci}")
                        nc.vector.reciprocal(rec[dsl, :], avs[ci][dsl, :])
                        nc.vector.tensor_mul(
                            oc[hs:hs + 64, ch * 512:(ch + 1) * 512],
                            avs[ci][osl, :],
                            rec[dsl, :],
                        )
                        prog.add(("oc", b, hl, ch))
                    return fn

                return [norm(ci, ch) for ci, ch in enumerate(chunks)]

            # ---------------- static schedule ----------------
            load_x(0)
            load_x(1)
            alloc_batch(0)
            alloc_batch(1)

            def qkvtp_gens(b):
                return [
                    qkv_part(b, [(0, 0), (0, 1), (1, 0), (2, 0)]),
                    tpvo_part(b, range(0, 4)),
                    qkv_part(b, [(1, 1), (2, 1)]),
                    tpvo_part(b, range(4, 8)),
                    qkv_part(b, [(1, 2), (2, 2)]),
                    tpvo_part(b, range(8, 12)),
                    qkv_part(b, [(1, 3), (2, 3)]),
                    tpvo_part(b, range(12, 16)),
                    qkv_part(b, [(0, 2), (0, 3)]),
                ]

            fillers = qkvtp_gens(0) + qkvtp_gens(1)
            post = []
            for hl, qp in ((0, 0), (1, 0), (0, 1), (1, 1)):
                post = attn_block(0, hl, qp, fillers, 3, post)

            fillers.append(proj_part(0, range(0, 8)))
            fillers.append(proj_part(0, range(8, NT)))
            fillers.append(proj_part(1, range(0, 8)))
            for hl, qp in ((0, 0), (1, 0), (0, 1), (1, 1)):
                post = attn_block(1, hl, qp, fillers, 2, post)
            for fn in post:
                fn()
            drain_left = take(fillers, 10000)
            for _ in proj_part(1, range(8, NT)):
                pass
    nc.finalize()
    return nc


_NC = None


def _get_nc():
    global _NC
    if _NC is None:
        _NC = _build()
    return _NC


def _prep_shared(x):
    # x [B, N, C] -> xTr [B, 128, CT, N] bf16 (c = ct*128 + p)
    xT = x.transpose(0, 2, 1).reshape(B, CT, 128, N).transpose(0, 2, 1, 3)
    return np.ascontiguousarray(xT).astype(BF)


def kernel(x, w_qkv, w_proj, b_proj):
    x = np.asarray(x, dtype=np.float32)
    w_qkv = np.asarray(w_qkv, dtype=np.float32)
    w_proj = np.asarray(w_proj, dtype=np.float32)
    b_proj = np.asarray(b_proj, dtype=np.float32)

    xTr = _prep_shared(x)
    in_maps = []
    for core in range(NCORES):
        h0 = core * HPC
        rows = np.concatenate(
            [np.arange(h * D, (h + 1) * D) for h in range(h0, h0 + HPC)])
        wsel = np.concatenate(
            [w_qkv[rows, :], w_qkv[C + rows, :], w_qkv[2 * C + rows, :]], axis=0)
        # [384, C] -> [C, 384] -> [CT, 128, 3, 128] -> [128, CT, 3, 128]
        wTa = wsel.T.reshape(CT, 128, 3, 128).transpose(1, 0, 2, 3)
        wTa = np.ascontiguousarray(wTa).astype(BF)
        cols = np.arange(h0 * D, (h0 + HPC) * D)
        wpT = np.ascontiguousarray(w_proj[:, cols].T).astype(BF)
        in_maps.append({"xTr": xTr, "wT": wTa, "wpT": wpT})

    nc = _get_nc()
    res = run_bass_kernel_spmd(nc, in_maps, core_ids=list(range(NCORES)))
    out = np.zeros((B, N, C), dtype=np.float32)
    for core in range(NCORES):
        out += res.results[core]["y"]
    out += b_proj
    return out
